# revision 2
# baseline (speedup 1.0000x reference)
"""Contrastive-learning loss kernel for Trainium2 (8 NeuronCores, Bass/Tile).

Problem (hardcoded shapes): B=16, L=512, DIN1=256, DIN2=192, DH=256, DF=128.
  emb1 = MLP_a(feature1); emb2 = MLP_b(feature2)          # (B, L, DF)
  positive = rowdot(f1, f2) + band-mean terms              # (N,)  N = B*L = 8192
  negative = logsumexp(f1 @ f2.T, axis=-1) - log N         # (N,)
  loss = mean(-positive + negative)

Sharding: data-parallel over B for embeddings/positives (2 batches per core);
the N x N negatives matrix is sharded row-wise. Each core computes the full
emb2 from a column-ROTATED copy of feature2 (its own batches first), so the
device program is identical across cores (pure SPMD, no partition-id).

v2 design notes (ACT/exp is the bottleneck; ~55us/core floor at 1 elem/cyc/lane):
- All-bf16 PE path (weights, h, e) -> matmuls at full rate, LDW cheap/hidden.
- Phase 1 (cols 0:4096): np tiles [128,1024] f32 (2 banks, bufs=2) with FD=1024
  ACTIVATE(Exp, accum_out) while MLP2 chunks stream in the other 4 banks.
- Phase 2 (cols 4096:8192): MLP2 done, PSUM pool swapped for [128,2048] tiles
  (4 banks, bufs=2) -> FD=2048 ACTIVATEs halve the per-instruction overhead.
- Early exp-table load + warmup matmuls off the first-landed weight tile.
- Positives/bands in bf16 with fp32 reductions; identical value paths for the
  diagonal (pos row-dot vs sim) keep the -pos+lse cancellation bf16-consistent.

Outputs per core: pos_out (128, 8), se_out (128, 8) where column t holds
local rows [t*128, (t+1)*128). Host: loss = mean(-pos + log(se) - log N).
"""

import numpy as np

import concourse.bacc as bacc
import concourse.tile as tile
from concourse import mybir
from concourse.bass_utils import run_bass_kernel_spmd
from concourse.masks import make_identity

F32 = mybir.dt.float32
BF16 = mybir.dt.bfloat16

B, L, DIN1, DIN2, DH, DF = 16, 512, 256, 192, 256, 128
N = B * L            # 8192 total rows
NCORES = 8
NB = B // NCORES     # 2 local batches per core
NLOC = NB * L        # 1024 local rows per core
NT = NLOC // 128     # 8 local row tiles
NG1 = 4              # phase-1 groups of 1024 cols (cols 0:4096)
NS2 = 2              # phase-2 supergroups of 2048 cols (cols 4096:8192)
NACC = NG1 + NS2     # accumulator columns per row tile


def _build(share_tgt: bool):
    nc = bacc.Bacc("TRN2", target_bir_lowering=False, debug=False)

    x1t_d = nc.dram_tensor("x1t", [DIN1, NLOC], BF16, kind="ExternalInput")
    x2t_d = nc.dram_tensor("x2t", [DIN2, N], BF16, kind="ExternalInput")
    w1a_d = nc.dram_tensor("w1a", [DIN1, DH], BF16, kind="ExternalInput")
    w2a_d = nc.dram_tensor("w2a", [DH, DF], BF16, kind="ExternalInput")
    w1b_d = nc.dram_tensor("w1b", [DIN2, DH], BF16, kind="ExternalInput")
    w2b_d = nc.dram_tensor("w2b", [DH, DF], BF16, kind="ExternalInput")
    bpk_d = nc.dram_tensor("bpk", [128, 6], F32, kind="ExternalInput")
    bms_d = nc.dram_tensor("bms", [L, L], BF16, kind="ExternalInput")
    cis_d = nc.dram_tensor("cis", [128, NT], F32, kind="ExternalInput")
    if not share_tgt:
        bmt_d = nc.dram_tensor("bmt", [L, L], BF16, kind="ExternalInput")
        cit_d = nc.dram_tensor("cit", [128, NT], F32, kind="ExternalInput")
    pos_d = nc.dram_tensor("pos_out", [128, NT], F32, kind="ExternalOutput")
    se_d = nc.dram_tensor("se_out", [128, NT], F32, kind="ExternalOutput")

    with tile.TileContext(nc) as tc:
        import contextlib

        with contextlib.ExitStack() as stack:
            const = stack.enter_context(tc.tile_pool(name="const", bufs=1))
            big = stack.enter_context(tc.tile_pool(name="big", bufs=1))
            h2pool = stack.enter_context(tc.tile_pool(name="h2pool", bufs=3))
            posp = stack.enter_context(tc.tile_pool(name="posp", bufs=2))

            # ---- pull the exp table into ACT ASAP (2.7us load hides in head)
            tl0 = const.tile([128, 8], F32)
            nc.gpsimd.memset(tl0[:], 0.0)
            tl1 = const.tile([128, 8], F32)
            nc.scalar.activation(
                out=tl1[:], in_=tl0[:], func=mybir.ActivationFunctionType.Exp
            )

            # ---- DMA issues, priority order (each dma_start ~0.6us on Sync)
            w1a = const.tile([128, 2, DH], BF16)
            nc.sync.dma_start(
                out=w1a[:], in_=w1a_d.ap().rearrange("(t p) m -> p t m", p=128)
            )
            # feature2^T (bf16), group-0 (local) columns first
            x2a = big.tile([128, N], BF16)
            x2b = big.tile([64, N], BF16)
            g0 = slice(0, 2048)
            nc.sync.dma_start(out=x2a[:, g0], in_=x2t_d.ap()[0:128, g0])
            nc.sync.dma_start(out=x2b[:, g0], in_=x2t_d.ap()[128:DIN2, g0])
            x1t = big.tile([128, 2, NLOC], BF16)
            for cc in range(2):
                nc.sync.dma_start(
                    out=x1t[:, :, cc * 512 : (cc + 1) * 512],
                    in_=x1t_d.ap().rearrange("(t p) c -> p t c", p=128)[
                        :, :, cc * 512 : (cc + 1) * 512
                    ],
                )
            w1b_a = const.tile([128, DH], BF16)
            nc.sync.dma_start(out=w1b_a[:], in_=w1b_d.ap()[0:128, :])
            w1b_b = const.tile([64, DH], BF16)
            nc.sync.dma_start(out=w1b_b[:], in_=w1b_d.ap()[128:DIN2, :])
            bpk = const.tile([128, 6], F32)
            nc.sync.dma_start(out=bpk[:], in_=bpk_d.ap())
            w2a = const.tile([128, 2, DF], BF16)
            nc.sync.dma_start(
                out=w2a[:], in_=w2a_d.ap().rearrange("(t p) m -> p t m", p=128)
            )
            w2b = const.tile([128, 2, DF], BF16)
            nc.sync.dma_start(
                out=w2b[:], in_=w2b_d.ap().rearrange("(t p) m -> p t m", p=128)
            )
            cis = const.tile([128, NT], F32)
            nc.sync.dma_start(out=cis[:], in_=cis_d.ap())
            for g in range(1, 4):
                cs = slice(g * 2048, (g + 1) * 2048)
                nc.sync.dma_start(out=x2a[:, cs], in_=x2t_d.ap()[0:128, cs])
                nc.sync.dma_start(out=x2b[:, cs], in_=x2t_d.ap()[128:DIN2, cs])
            bms = const.tile([128, 4, L], BF16)
            nc.sync.dma_start(
                out=bms[:], in_=bms_d.ap().rearrange("(t p) j -> p t j", p=128)
            )
            if share_tgt:
                bmt, cit = bms, cis
            else:
                bmt = const.tile([128, 4, L], BF16)
                nc.sync.dma_start(
                    out=bmt[:], in_=bmt_d.ap().rearrange("(t p) j -> p t j", p=128)
                )
                cit = const.tile([128, NT], F32)
                nc.sync.dma_start(out=cit[:], in_=cit_d.ap())

            b1a, b2a = bpk[:, 0:2], bpk[:, 2:3]
            b1b, b2b = bpk[:, 3:5], bpk[:, 5:6]

            # bf16 identity for PE transposes
            identf = const.tile([128, 128], F32)
            make_identity(nc, identf[:])
            ident = const.tile([128, 128], BF16)
            nc.vector.tensor_copy(ident[:], identf[:])

            e1t = big.tile([128, NLOC], BF16)
            e2t = big.tile([128, N], BF16)
            h1t = big.tile([128, 2, NLOC], BF16)
            e1nat = big.tile([128, NT, DF], BF16)
            e2nat = big.tile([128, NT, DF], BF16)
            w1nat = big.tile([128, NT, DF], BF16)
            w2snat = big.tile([128, NT, DF], BF16)
            w2tnat = w2snat if share_tgt else big.tile([128, NT, DF], BF16)
            pos_all = big.tile([128, NT], F32)
            acc_all = big.tile([128, NT * NACC], F32)
            se_all = big.tile([128, NT], F32)

            # ================= PHASE 1: MLP1/MLP2 + cols 0:4096 =================
            with tc.tile_pool(name="ps1", bufs=1, space="PSUM") as ps1:
                # PE warm-up off w1a (first tile to land); HAM -> 8/8
                warm_ps = ps1.tile([128, DH], F32, tag="sps", bufs=2)
                for i in range(12):
                    nc.tensor.matmul(
                        warm_ps[:],
                        w1a[:, i % 2, 0:128],
                        w1a[:, (i + 1) % 2, :],
                        start=True,
                        stop=True,
                    )

                # MLP1: h1 = relu(W1a^T x1 + b1a); e1 = W2a^T h1 + b2a
                for cc in range(2):
                    cols = slice(cc * 512, (cc + 1) * 512)
                    h1ps = ps1.tile([128, 2, 512], F32, tag="hps", bufs=1)
                    for mt in range(2):
                        for kt in range(2):
                            nc.tensor.matmul(
                                h1ps[:, mt, :],
                                w1a[:, kt, mt * 128 : (mt + 1) * 128],
                                x1t[:, kt, cols],
                                start=(kt == 0),
                                stop=(kt == 1),
                            )
                    for mt in range(2):
                        nc.vector.tensor_scalar(
                            out=h1t[:, mt, cols],
                            in0=h1ps[:, mt, :],
                            scalar1=b1a[:, mt : mt + 1],
                            scalar2=0.0,
                            op0=mybir.AluOpType.add,
                            op1=mybir.AluOpType.max,
                        )
                    e1ps = ps1.tile([128, 512], F32, tag="sps", bufs=2)
                    for kt in range(2):
                        nc.tensor.matmul(
                            e1ps[:],
                            w2a[:, kt, :],
                            h1t[:, kt, cols],
                            start=(kt == 0),
                            stop=(kt == 1),
                        )
                    nc.vector.tensor_scalar_add(
                        out=e1t[:, cols], in0=e1ps[:], scalar1=b2a
                    )

                def mlp2_chunk(ct):
                    cols = slice(ct * 512, (ct + 1) * 512)
                    h2ps = ps1.tile(
                        [128, 2, 512], F32, tag="hps", bufs=1, name=f"h2ps{ct}"
                    )
                    for mt in range(2):
                        msl = slice(mt * 128, (mt + 1) * 128)
                        nc.tensor.matmul(
                            h2ps[:, mt, :],
                            w1b_a[:, msl],
                            x2a[:, cols],
                            start=True,
                            stop=False,
                        )
                        nc.tensor.matmul(
                            h2ps[:, mt, :],
                            w1b_b[:, msl],
                            x2b[:, cols],
                            start=False,
                            stop=True,
                        )
                    h2t = h2pool.tile([128, 2, 512], BF16, tag="h2t", name=f"h2t{ct}")
                    for mt in range(2):
                        nc.vector.tensor_scalar(
                            out=h2t[:, mt, :],
                            in0=h2ps[:, mt, :],
                            scalar1=b1b[:, mt : mt + 1],
                            scalar2=0.0,
                            op0=mybir.AluOpType.add,
                            op1=mybir.AluOpType.max,
                        )
                    e2ps = ps1.tile([128, 512], F32, tag="sps", bufs=2, name=f"e2ps{ct}")
                    for kt in range(2):
                        nc.tensor.matmul(
                            e2ps[:],
                            w2b[:, kt, :],
                            h2t[:, kt, :],
                            start=(kt == 0),
                            stop=(kt == 1),
                        )
                    nc.vector.tensor_scalar_add(
                        out=e2t[:, cols], in0=e2ps[:], scalar1=b2b
                    )

                mlp2_chunk(0)
                mlp2_chunk(1)

                def transpose_to(dst, srcT, t):
                    tp = ps1.tile([128, 128], BF16, tag="sps", bufs=2, name=f"tp{t}")
                    nc.tensor.transpose(
                        tp[:], srcT[:, t * 128 : (t + 1) * 128], ident[:]
                    )
                    nc.vector.tensor_copy(dst[:, t, :], tp[:])

                # banded sums: W[j,:] = sum_{|m-j|<=r} e[m,:]; bm stationary is
                # reused across the two local batches (one LDW per (jt, mt)).
                def band(dst, bm, src):
                    for jt in range(4):
                        wps = [
                            ps1.tile(
                                [128, 128], F32, tag="sps", bufs=2, name=f"wps{jt}{b}"
                            )
                            for b in range(NB)
                        ]
                        for mt in range(4):
                            for b in range(NB):
                                nc.tensor.matmul(
                                    wps[b][:],
                                    bm[:, mt, jt * 128 : (jt + 1) * 128],
                                    src[:, 4 * b + mt, :],
                                    start=(mt == 0),
                                    stop=(mt == 3),
                                )
                        for b in range(NB):
                            nc.vector.tensor_copy(dst[:, 4 * b + jt, :], wps[b][:])

                def neg_tile_1024(t, g):
                    lhs = e1t[:, t * 128 : (t + 1) * 128]
                    np_ps = ps1.tile([128, 1024], F32, tag="neg", bufs=2)
                    for i in range(2):
                        c0 = g * 1024 + i * 512
                        nc.tensor.matmul(
                            np_ps[:, i * 512 : (i + 1) * 512],
                            lhs,
                            e2t[:, c0 : c0 + 512],
                            start=True,
                            stop=True,
                        )
                    idx = t * NACC + g
                    nc.scalar.activation(
                        out=np_ps[:],
                        in_=np_ps[:],
                        func=mybir.ActivationFunctionType.Exp,
                        accum_out=acc_all[:, idx : idx + 1],
                    )

                # g=0 negatives interleaved with e-side transposes/bands
                for t in range(NT):
                    transpose_to(e1nat, e1t, t)
                neg_tile_1024(0, 0)
                neg_tile_1024(1, 0)
                for t in range(NT):
                    transpose_to(e2nat, e2t, t)
                neg_tile_1024(2, 0)
                neg_tile_1024(3, 0)
                band(w1nat, bms, e1nat)
                neg_tile_1024(4, 0)
                neg_tile_1024(5, 0)
                band(w2snat, bms, e2nat)
                if not share_tgt:
                    band(w2tnat, bmt, e2nat)
                neg_tile_1024(6, 0)
                neg_tile_1024(7, 0)

                mlp2_chunk(2)
                mlp2_chunk(3)
                mlp2_chunk(4)
                mlp2_chunk(5)
                for t in range(NT):
                    neg_tile_1024(t, 1)
                    if t < 4:
                        mlp2_chunk(6 + t)
                # positives on DVE (overlaps ACT); needs bands + e-nat tiles
                for b in range(NB):
                    bsl = slice(4 * b, 4 * b + 4)
                    ga = posp.tile([128, 4, DF], BF16, tag="posg")
                    r1 = posp.tile([128, 4], F32, tag="post")
                    r2 = posp.tile([128, 4], F32, tag="post")
                    if share_tgt:
                        nc.vector.tensor_add(ga[:], w1nat[:, bsl, :], w2snat[:, bsl, :])
                        nc.vector.tensor_mul(ga[:], ga[:], e1nat[:, bsl, :])
                    else:
                        nc.vector.tensor_mul(ga[:], w1nat[:, bsl, :], e1nat[:, bsl, :])
                    nc.vector.tensor_reduce(
                        out=r1[:],
                        in_=ga[:],
                        axis=mybir.AxisListType.X,
                        op=mybir.AluOpType.add,
                    )
                    gb = posp.tile([128, 4, DF], BF16, tag="posg")
                    nc.vector.tensor_mul(gb[:], w2snat[:, bsl, :], e2nat[:, bsl, :])
                    nc.vector.tensor_reduce(
                        out=r2[:],
                        in_=gb[:],
                        axis=mybir.AxisListType.X,
                        op=mybir.AluOpType.add,
                    )
                    nc.vector.tensor_add(r1[:], r1[:], r2[:])
                    nc.vector.tensor_mul(r1[:], r1[:], cis[:, bsl])
                    if not share_tgt:
                        gc = posp.tile([128, 4, DF], BF16, tag="posg")
                        nc.vector.tensor_mul(gc[:], w2tnat[:, bsl, :], e1nat[:, bsl, :])
                        rt = posp.tile([128, 4], F32, tag="post")
                        nc.vector.tensor_reduce(
                            out=rt[:],
                            in_=gc[:],
                            axis=mybir.AxisListType.X,
                            op=mybir.AluOpType.add,
                        )
                        nc.vector.tensor_mul(rt[:], rt[:], cit[:, bsl])
                        nc.vector.tensor_add(r1[:], r1[:], rt[:])
                    gd = posp.tile([128, 4, DF], BF16, tag="posg")
                    nc.vector.tensor_mul(gd[:], e1nat[:, bsl, :], e2nat[:, bsl, :])
                    r3 = posp.tile([128, 4], F32, tag="post")
                    nc.vector.tensor_reduce(
                        out=r3[:],
                        in_=gd[:],
                        axis=mybir.AxisListType.X,
                        op=mybir.AluOpType.add,
                    )
                    nc.vector.tensor_add(pos_all[:, bsl], r1[:], r3[:])
                nc.sync.dma_start(out=pos_d.ap(), in_=pos_all[:])

                for t in range(NT):
                    neg_tile_1024(t, 2)
                    if t < 4:
                        mlp2_chunk(10 + t)
                for t in range(NT):
                    neg_tile_1024(t, 3)
                    if t < 2:
                        mlp2_chunk(14 + t)

            # ================= PHASE 2: cols 4096:8192, FD=2048 =================
            with tc.tile_pool(name="ps2", bufs=1, space="PSUM") as ps2:
                for s in range(NS2):
                    for t in range(NT):
                        lhs = e1t[:, t * 128 : (t + 1) * 128]
                        np2 = ps2.tile([128, 2048], F32, tag="neg2", bufs=2)
                        for i in range(4):
                            c0 = 4096 + s * 2048 + i * 512
                            nc.tensor.matmul(
                                np2[:, i * 512 : (i + 1) * 512],
                                lhs,
                                e2t[:, c0 : c0 + 512],
                                start=True,
                                stop=True,
                            )
                        idx = t * NACC + NG1 + s
                        nc.scalar.activation(
                            out=np2[:],
                            in_=np2[:],
                            func=mybir.ActivationFunctionType.Exp,
                            accum_out=acc_all[:, idx : idx + 1],
                        )

            nc.vector.tensor_reduce(
                out=se_all[:],
                in_=acc_all[:].rearrange("p (t g) -> p t g", t=NT),
                axis=mybir.AxisListType.X,
                op=mybir.AluOpType.add,
            )
            nc.sync.dma_start(out=se_d.ap(), in_=se_all[:])

    nc.compile()
    return nc


_BUILD_CACHE: dict = {}


def _get_nc(share_tgt: bool):
    if share_tgt not in _BUILD_CACHE:
        _BUILD_CACHE[share_tgt] = _build(share_tgt)
    return _BUILD_CACHE[share_tgt]


def _band_mask(r: int) -> np.ndarray:
    """mask[m, j] = 1 if |m-j| <= r (and inside [0,L)) else 0."""
    bm = np.zeros((L, L), dtype=np.float32)
    if r > 0:
        j = np.arange(L)
        lo = np.maximum(j - r, 0)
        hi = np.minimum(j + r + 1, L)
        m = np.arange(L)[:, None]
        bm = ((m >= lo[None, :]) & (m < hi[None, :])).astype(np.float32)
    return bm


def _cnt_inv(r: int) -> np.ndarray:
    """(128, NT) tile of 1/count(j) per local row (j = row mod L)."""
    j = np.arange(L)
    if r > 0:
        cnt = (np.minimum(j + r + 1, L) - np.maximum(j - r, 0)).astype(np.float64)
    else:
        cnt = np.ones(L)
    cinv = (1.0 / cnt).astype(np.float32)
    rows = (np.arange(NLOC) % L)
    return np.ascontiguousarray(cinv[rows].reshape(NT, 128).T)


def kernel(**inputs):
    loss, _ = _run(inputs, trace=False)
    return loss


def _run(inputs, trace=False, trace_kwargs=None):
    import ml_dtypes

    bf16 = ml_dtypes.bfloat16
    feature1 = inputs["feature1"]
    feature2 = inputs["feature2"]
    W1a, b1a, W2a, b2a = inputs["W1a"], inputs["b1a"], inputs["W2a"], inputs["b2a"]
    W1b, b1b, W2b, b2b = inputs["W1b"], inputs["b1b"], inputs["W2b"], inputs["b2b"]
    f1 = np.ascontiguousarray(np.asarray(feature1, dtype=np.float32))
    f2 = np.ascontiguousarray(np.asarray(feature2, dtype=np.float32))
    r_self = int(np.asarray(inputs["positive_range_self"]))
    r_tgt = int(np.asarray(inputs["positive_range_tgt"]))
    share_tgt = r_tgt == r_self

    nc = _get_nc(share_tgt)

    x2t_full = np.ascontiguousarray(f2.reshape(N, DIN2).T.astype(bf16))  # (192, 8192)
    bpk = np.zeros((128, 6), dtype=np.float32)
    bpk[:, 0:2] = np.asarray(b1a, np.float32).reshape(2, 128).T
    bpk[:, 2] = np.asarray(b2a, np.float32)
    bpk[:, 3:5] = np.asarray(b1b, np.float32).reshape(2, 128).T
    bpk[:, 5] = np.asarray(b2b, np.float32)
    common = {
        "w1a": np.ascontiguousarray(np.asarray(W1a, np.float32).astype(bf16)),
        "w2a": np.ascontiguousarray(np.asarray(W2a, np.float32).astype(bf16)),
        "w1b": np.ascontiguousarray(np.asarray(W1b, np.float32).astype(bf16)),
        "w2b": np.ascontiguousarray(np.asarray(W2b, np.float32).astype(bf16)),
        "bpk": bpk,
        "bms": _band_mask(r_self).astype(bf16),
        "cis": _cnt_inv(r_self),
    }
    if not share_tgt:
        common["bmt"] = _band_mask(r_tgt).astype(bf16)
        common["cit"] = _cnt_inv(r_tgt)

    in_maps = []
    for c in range(NCORES):
        x1t = np.ascontiguousarray(
            f1[c * NB : (c + 1) * NB].reshape(NLOC, DIN1).T.astype(bf16)
        )  # (256, 1024)
        # rotate feature2^T columns so this core's rows come first
        x2t = np.ascontiguousarray(
            np.concatenate(
                [x2t_full[:, c * NLOC :], x2t_full[:, : c * NLOC]], axis=1
            )
        )
        in_maps.append({**common, "x1t": x1t, "x2t": x2t})

    res = run_bass_kernel_spmd(
        nc,
        in_maps,
        core_ids=list(range(NCORES)),
        trace=trace,
        **(trace_kwargs or {}),
    )

    pos = np.empty(N, dtype=np.float64)
    se = np.empty(N, dtype=np.float64)
    for c in range(NCORES):
        # column t holds local rows [t*128, (t+1)*128) in partitions
        p = res.results[c]["pos_out"]  # (128, NT)
        s = res.results[c]["se_out"]
        pos[c * NLOC : (c + 1) * NLOC] = p.T.reshape(NLOC)
        se[c * NLOC : (c + 1) * NLOC] = s.T.reshape(NLOC)

    neg = np.log(se) - np.log(float(N))
    loss = np.mean(-pos + neg)
    return np.array(loss, dtype=np.float32), res


# revision 4
# speedup vs baseline: 1.1852x; 1.1852x over previous
"""Contrastive-learning loss kernel for Trainium2 (8 NeuronCores, Bass/Tile).

Problem (hardcoded shapes): B=16, L=512, DIN1=256, DIN2=192, DH=256, DF=128.
  emb1 = MLP_a(feature1); emb2 = MLP_b(feature2)          # (B, L, DF)
  positive = rowdot(f1, f2) + band-mean terms              # (N,)  N = B*L = 8192
  negative = logsumexp(f1 @ f2.T, axis=-1) - log N         # (N,)
  loss = mean(-positive + negative)

Sharding: data-parallel over B (2 batches/core); the N x N negatives matrix is
sharded row-wise; each core computes full emb2 from a column-rotated feature2
copy (pure SPMD, no partition-id).

v3 design (ACT/exp is the bottleneck: 8.4M exps/core = ~55us at 1/cyc/lane):
- all-bf16 PE path; np sim tiles fp32 (TRN2 matmul only writes fp32 PSUM).
- phase 1 (cols 0:4096): FD=1024 ACTIVATE(Exp, accum) tiles [128,1024] (2
  banks, bufs=2) while 1024-col MLP2 chunks stream through 1-bank sps tiles
  (bufs=4). MLP2 work is emitted as small "pieces" paced between np tiles so
  the PE FIFO never starves the ACT engine.
- phase 2 (cols 4096:8192): PSUM pool swapped to [128,2048] tiles (bufs=2)
  -> FD=2048 halves the per-instruction ACT overhead.
- band sums via DVE prefix-scan + shifted differences (exactly the reference's
  cumsum trick) - zero PE matmuls, no mask DMAs; runs in phase 2's DVE shadow.
- e1nat/e2nat/W-nat transposes via the DMA xbar (dma_start_transpose) - zero
  PE/PSUM cost.
- positives on DVE in phase 2's shadow; exp table preloaded at t~0.

Outputs per core: pos_out (128, 8), se_out (128, 8); col t = local rows
[t*128,(t+1)*128). Host: loss = mean(-pos + log(se) - log N).
"""

import numpy as np

import concourse.bacc as bacc
import concourse.tile as tile
from concourse import mybir
from concourse.bass_utils import run_bass_kernel_spmd

F32 = mybir.dt.float32
BF16 = mybir.dt.bfloat16

B, L, DIN1, DIN2, DH, DF = 16, 512, 256, 192, 256, 128
N = B * L            # 8192 total rows
NCORES = 8
NB = B // NCORES     # 2 local batches per core
NLOC = NB * L        # 1024 local rows per core
NT = NLOC // 128     # 8 local row tiles
NG1 = 4              # phase-1 groups of 1024 cols (cols 0:4096)
NS2 = 2              # phase-2 supergroups of 2048 cols (cols 4096:8192)
NACC = NG1 + NS2     # accumulator columns per row tile


def _build(r_self: int, r_tgt: int):
    share_tgt = r_tgt == r_self
    has_self = r_self > 0
    has_tgt = r_tgt > 0

    nc = bacc.Bacc("TRN2", target_bir_lowering=False, debug=False)

    x1t_d = nc.dram_tensor("x1t", [DIN1, NLOC], BF16, kind="ExternalInput")
    x2t_d = nc.dram_tensor("x2t", [DIN2, N], BF16, kind="ExternalInput")
    w1a_d = nc.dram_tensor("w1a", [DIN1, DH], BF16, kind="ExternalInput")
    w2a_d = nc.dram_tensor("w2a", [DH, DF], BF16, kind="ExternalInput")
    w1b_d = nc.dram_tensor("w1b", [DIN2, DH], BF16, kind="ExternalInput")
    w2b_d = nc.dram_tensor("w2b", [DH, DF], BF16, kind="ExternalInput")
    bpk_d = nc.dram_tensor("bpk", [128, 6], F32, kind="ExternalInput")
    cis_d = nc.dram_tensor("cis", [128, NT], F32, kind="ExternalInput")
    cit_d = None
    if has_tgt and not share_tgt:
        cit_d = nc.dram_tensor("cit", [128, NT], F32, kind="ExternalInput")
    pos_d = nc.dram_tensor("pos_out", [128, NT], F32, kind="ExternalOutput")
    se_d = nc.dram_tensor("se_out", [128, NT], F32, kind="ExternalOutput")

    with tile.TileContext(nc) as tc:
        import contextlib

        with contextlib.ExitStack() as stack:
            const = stack.enter_context(tc.tile_pool(name="const", bufs=1))
            big = stack.enter_context(tc.tile_pool(name="big", bufs=1))
            h2pool = stack.enter_context(tc.tile_pool(name="h2pool", bufs=3))
            posp = stack.enter_context(tc.tile_pool(name="posp", bufs=2))

            # pull the exp table into ACT immediately (2.7us hides in the head)
            tl0 = const.tile([128, 8], F32)
            nc.gpsimd.memset(tl0[:], 0.0)
            tl1 = const.tile([128, 8], F32)
            nc.scalar.activation(
                out=tl1[:], in_=tl0[:], func=mybir.ActivationFunctionType.Exp
            )

            # ---- DMA issues in priority order (x1t first: MLP1 is the head)
            x1t = big.tile([128, 2, NLOC], BF16)
            for cc in range(2):
                nc.sync.dma_start(
                    out=x1t[:, :, cc * 512 : (cc + 1) * 512],
                    in_=x1t_d.ap().rearrange("(t p) c -> p t c", p=128)[
                        :, :, cc * 512 : (cc + 1) * 512
                    ],
                )
            w1a = const.tile([128, 2, DH], BF16)
            nc.sync.dma_start(
                out=w1a[:], in_=w1a_d.ap().rearrange("(t p) m -> p t m", p=128)
            )
            bpk = const.tile([128, 6], F32)
            nc.sync.dma_start(out=bpk[:], in_=bpk_d.ap())
            w2a = const.tile([128, 2, DF], BF16)
            nc.sync.dma_start(
                out=w2a[:], in_=w2a_d.ap().rearrange("(t p) m -> p t m", p=128)
            )
            x2a = big.tile([128, N], BF16)
            x2b = big.tile([64, N], BF16)
            g0 = slice(0, 2048)
            nc.sync.dma_start(out=x2a[:, g0], in_=x2t_d.ap()[0:128, g0])
            nc.sync.dma_start(out=x2b[:, g0], in_=x2t_d.ap()[128:DIN2, g0])
            w1b_a = const.tile([128, DH], BF16)
            nc.sync.dma_start(out=w1b_a[:], in_=w1b_d.ap()[0:128, :])
            w1b_b = const.tile([64, DH], BF16)
            nc.sync.dma_start(out=w1b_b[:], in_=w1b_d.ap()[128:DIN2, :])
            w2b = const.tile([128, 2, DF], BF16)
            nc.sync.dma_start(
                out=w2b[:], in_=w2b_d.ap().rearrange("(t p) m -> p t m", p=128)
            )
            cis = const.tile([128, NT], F32)
            nc.sync.dma_start(out=cis[:], in_=cis_d.ap())
            if cit_d is not None:
                cit = const.tile([128, NT], F32)
                nc.sync.dma_start(out=cit[:], in_=cit_d.ap())
            else:
                cit = cis
            for g in range(1, 4):
                cs = slice(g * 2048, (g + 1) * 2048)
                nc.sync.dma_start(out=x2a[:, cs], in_=x2t_d.ap()[0:128, cs])
                nc.sync.dma_start(out=x2b[:, cs], in_=x2t_d.ap()[128:DIN2, cs])

            b1a, b2a = bpk[:, 0:2], bpk[:, 2:3]
            b1b, b2b = bpk[:, 3:5], bpk[:, 5:6]

            e1t = big.tile([128, NLOC], BF16)
            e2t = big.tile([128, N], BF16)
            h1t = big.tile([128, 2, NLOC], BF16)
            e1nat = big.tile([128, NT, DF], BF16)
            e2nat = big.tile([128, NT, DF], BF16)
            pos_all = big.tile([128, NT], F32)
            acc_all = big.tile([128, NT * NACC], F32)
            se_all = big.tile([128, NT], F32)
            # prefix sums + band-sum (T layout) scratch
            s1S = big.tile([128, NB, L], F32)
            s2S = big.tile([128, NB, L], F32)
            w1T = big.tile([128, NB, L], BF16)
            w2sT = big.tile([128, NB, L], BF16)
            w2tT = w2sT if share_tgt else big.tile([128, NB, L], BF16)
            w1nat = big.tile([128, NT, DF], BF16)
            w2snat = big.tile([128, NT, DF], BF16)
            w2tnat = w2snat if share_tgt else big.tile([128, NT, DF], BF16)

            # ================= PHASE 1: MLPs + negs cols 0:4096 =================
            with tc.tile_pool(name="ps1", bufs=1, space="PSUM") as ps1:

                def mlp1_h(cc, mt):
                    cols = slice(cc * 512, (cc + 1) * 512)
                    hp = ps1.tile([128, 512], F32, tag="sps", bufs=4, name=f"h1p{cc}{mt}")
                    for kt in range(2):
                        nc.tensor.matmul(
                            hp[:],
                            w1a[:, kt, mt * 128 : (mt + 1) * 128],
                            x1t[:, kt, cols],
                            start=(kt == 0),
                            stop=(kt == 1),
                        )
                    nc.vector.tensor_scalar(
                        out=h1t[:, mt, cols],
                        in0=hp[:],
                        scalar1=b1a[:, mt : mt + 1],
                        scalar2=0.0,
                        op0=mybir.AluOpType.add,
                        op1=mybir.AluOpType.max,
                    )

                def mlp1_e(cc):
                    cols = slice(cc * 512, (cc + 1) * 512)
                    ep = ps1.tile([128, 512], F32, tag="sps", bufs=4, name=f"e1p{cc}")
                    for kt in range(2):
                        nc.tensor.matmul(
                            ep[:],
                            w2a[:, kt, :],
                            h1t[:, kt, cols],
                            start=(kt == 0),
                            stop=(kt == 1),
                        )
                    nc.vector.tensor_scalar_add(
                        out=e1t[:, cols], in0=ep[:], scalar1=b2a
                    )

                # MLP2 chunk ct covers cols [ct*1024, (ct+1)*1024): emitted as
                # 6 pieces (4x h-half, 2x e2-half) paced between np tiles.
                def chunk_h(ct, mt, half):
                    cols = slice(ct * 1024 + half * 512, ct * 1024 + (half + 1) * 512)
                    hp = ps1.tile(
                        [128, 512], F32, tag="sps", bufs=4, name=f"h2p{ct}{mt}{half}"
                    )
                    msl = slice(mt * 128, (mt + 1) * 128)
                    nc.tensor.matmul(
                        hp[:], w1b_a[:, msl], x2a[:, cols], start=True, stop=False
                    )
                    nc.tensor.matmul(
                        hp[:], w1b_b[:, msl], x2b[:, cols], start=False, stop=True
                    )
                    h2t = _h2t[ct]
                    nc.vector.tensor_scalar(
                        out=h2t[:, mt, half * 512 : (half + 1) * 512],
                        in0=hp[:],
                        scalar1=b1b[:, mt : mt + 1],
                        scalar2=0.0,
                        op0=mybir.AluOpType.add,
                        op1=mybir.AluOpType.max,
                    )

                def chunk_e2(ct, half):
                    hsl = slice(half * 512, (half + 1) * 512)
                    cols = slice(ct * 1024 + half * 512, ct * 1024 + (half + 1) * 512)
                    ep = ps1.tile(
                        [128, 512], F32, tag="sps", bufs=4, name=f"e2p{ct}{half}"
                    )
                    h2t = _h2t[ct]
                    for kt in range(2):
                        nc.tensor.matmul(
                            ep[:],
                            w2b[:, kt, :],
                            h2t[:, kt, hsl],
                            start=(kt == 0),
                            stop=(kt == 1),
                        )
                    nc.vector.tensor_scalar_add(
                        out=e2t[:, cols], in0=ep[:], scalar1=b2b
                    )

                _h2t = {}

                def chunk_pieces(ct):
                    _h2t[ct] = h2pool.tile(
                        [128, 2, 1024], BF16, tag="h2t", name=f"h2t{ct}"
                    )
                    return [
                        lambda: chunk_h(ct, 0, 0),
                        lambda: chunk_h(ct, 0, 1),
                        lambda: chunk_h(ct, 1, 0),
                        lambda: chunk_h(ct, 1, 1),
                        lambda: chunk_e2(ct, 0),
                        lambda: chunk_e2(ct, 1),
                    ]

                def neg_tile_1024(t, g):
                    lhs = e1t[:, t * 128 : (t + 1) * 128]
                    np_ps = ps1.tile([128, 1024], F32, tag="neg", bufs=2)
                    for i in range(2):
                        c0 = g * 1024 + i * 512
                        nc.tensor.matmul(
                            np_ps[:, i * 512 : (i + 1) * 512],
                            lhs,
                            e2t[:, c0 : c0 + 512],
                            start=True,
                            stop=True,
                        )
                    idx = t * NACC + g
                    nc.scalar.activation(
                        out=np_ps[:],
                        in_=np_ps[:],
                        func=mybir.ActivationFunctionType.Exp,
                        accum_out=acc_all[:, idx : idx + 1],
                    )

                # head: MLP1 cc0 + chunk0, then negs start; MLP1 cc1 paced in
                mlp1_h(0, 0)
                mlp1_h(0, 1)
                mlp1_e(0)
                for p in chunk_pieces(0):
                    p()

                def mlp1_cc1():
                    mlp1_h(1, 0)
                    mlp1_h(1, 1)
                    mlp1_e(1)

                # e1nat via DMA xbar transpose (no PE/PSUM cost)
                def e1_tps():
                    for t in range(NT):
                        nc.sync.dma_start_transpose(
                            out=e1nat[:, t, :], in_=e1t[:, t * 128 : (t + 1) * 128]
                        )

                def e2_tps():
                    for t in range(NT):
                        nc.sync.dma_start_transpose(
                            out=e2nat[:, t, :], in_=e2t[:, t * 128 : (t + 1) * 128]
                        )

                # pacing: era g runs np tiles t=0..7 with pieces of chunks
                # g+1, g+2 (2 pieces after each of t=0..5)
                for g in range(NG1):
                    pieces = []
                    if g == 0:
                        pieces += [mlp1_cc1]
                        pieces += chunk_pieces(1)
                        pieces += chunk_pieces(2)
                    elif g == 1:
                        pieces += chunk_pieces(3)
                        pieces += chunk_pieces(4)
                    elif g == 2:
                        pieces += chunk_pieces(5)
                        pieces += chunk_pieces(6)
                    else:
                        pieces += chunk_pieces(7)
                        pieces += [e1_tps, e2_tps]
                    pi = 0
                    for t in range(NT):
                        neg_tile_1024(t, g)
                        take = 2 if t < 6 else 3
                        for _ in range(take):
                            if pi < len(pieces):
                                pieces[pi]()
                                pi += 1
                    while pi < len(pieces):
                        pieces[pi]()
                        pi += 1

                # ---- band sums via prefix scan (T layout), DVE-only ----
                def band_scan(S, src):
                    for b in range(NB):
                        seg = slice(b * L, (b + 1) * L)
                        nc.vector.tensor_tensor_scan(
                            out=S[:, b, :],
                            data0=src[:, seg],
                            data1=src[:, seg],
                            initial=0.0,
                            op0=mybir.AluOpType.add,
                            op1=mybir.AluOpType.bypass,
                        )

                def band_from_scan(WT, S, r):
                    # W[j] = S[min(j+r, L-1)] - (S[j-r-1] if j>=r+1 else 0)
                    for b in range(NB):
                        nc.vector.tensor_sub(
                            WT[:, b, r + 1 : L - r],
                            S[:, b, 2 * r + 1 : L],
                            S[:, b, 0 : L - 2 * r - 1],
                        )
                        nc.vector.tensor_copy(
                            WT[:, b, 0 : r + 1], S[:, b, r : 2 * r + 1]
                        )
                        nc.vector.tensor_scalar(
                            out=WT[:, b, L - r : L],
                            in0=S[:, b, L - 2 * r - 1 : L - r - 1],
                            scalar1=-1.0,
                            scalar2=S[:, b, L - 1 : L],
                            op0=mybir.AluOpType.mult,
                            op1=mybir.AluOpType.add,
                        )

                if has_self or has_tgt:
                    band_scan(s2S, e2t)
                if has_self:
                    band_scan(s1S, e1t)
                    band_from_scan(w1T, s1S, r_self)
                    band_from_scan(w2sT, s2S, r_self)
                if has_tgt and not share_tgt:
                    band_from_scan(w2tT, s2S, r_tgt)

                def w_tps(dst, WT):
                    for b in range(NB):
                        for jt in range(4):
                            nc.sync.dma_start_transpose(
                                out=dst[:, 4 * b + jt, :],
                                in_=WT[:, b, jt * 128 : (jt + 1) * 128],
                            )

                if has_self:
                    w_tps(w1nat, w1T)
                    w_tps(w2snat, w2sT)
                if has_tgt and not share_tgt:
                    w_tps(w2tnat, w2tT)

            # ================= PHASE 2: cols 4096:8192, FD=2048 =================
            with tc.tile_pool(name="ps2", bufs=1, space="PSUM") as ps2:

                def neg_tile_2048(t, s):
                    lhs = e1t[:, t * 128 : (t + 1) * 128]
                    np2 = ps2.tile([128, 2048], F32, tag="neg2", bufs=2)
                    for i in range(4):
                        c0 = 4096 + s * 2048 + i * 512
                        nc.tensor.matmul(
                            np2[:, i * 512 : (i + 1) * 512],
                            lhs,
                            e2t[:, c0 : c0 + 512],
                            start=True,
                            stop=True,
                        )
                    idx = t * NACC + NG1 + s
                    nc.scalar.activation(
                        out=np2[:],
                        in_=np2[:],
                        func=mybir.ActivationFunctionType.Exp,
                        accum_out=acc_all[:, idx : idx + 1],
                    )

                for t in range(NT):
                    neg_tile_2048(t, 0)

                # positives on DVE in the ACT shadow
                for b in range(NB):
                    bsl = slice(4 * b, 4 * b + 4)
                    r1 = posp.tile([128, 4], F32, tag="post")
                    have_r1 = False
                    if has_self:
                        ga = posp.tile([128, 4, DF], BF16, tag="posg")
                        r2 = posp.tile([128, 4], F32, tag="post")
                        if share_tgt:
                            nc.vector.tensor_add(
                                ga[:], w1nat[:, bsl, :], w2snat[:, bsl, :]
                            )
                            nc.vector.tensor_mul(ga[:], ga[:], e1nat[:, bsl, :])
                        else:
                            nc.vector.tensor_mul(
                                ga[:], w1nat[:, bsl, :], e1nat[:, bsl, :]
                            )
                        nc.vector.tensor_reduce(
                            out=r1[:],
                            in_=ga[:],
                            axis=mybir.AxisListType.X,
                            op=mybir.AluOpType.add,
                        )
                        gb = posp.tile([128, 4, DF], BF16, tag="posg")
                        nc.vector.tensor_mul(gb[:], w2snat[:, bsl, :], e2nat[:, bsl, :])
                        nc.vector.tensor_reduce(
                            out=r2[:],
                            in_=gb[:],
                            axis=mybir.AxisListType.X,
                            op=mybir.AluOpType.add,
                        )
                        nc.vector.tensor_add(r1[:], r1[:], r2[:])
                        nc.vector.tensor_mul(r1[:], r1[:], cis[:, bsl])
                        have_r1 = True
                    if has_tgt and not share_tgt:
                        gc = posp.tile([128, 4, DF], BF16, tag="posg")
                        nc.vector.tensor_mul(gc[:], w2tnat[:, bsl, :], e1nat[:, bsl, :])
                        rt = posp.tile([128, 4], F32, tag="post")
                        nc.vector.tensor_reduce(
                            out=rt[:],
                            in_=gc[:],
                            axis=mybir.AxisListType.X,
                            op=mybir.AluOpType.add,
                        )
                        nc.vector.tensor_mul(rt[:], rt[:], cit[:, bsl])
                        if have_r1:
                            nc.vector.tensor_add(r1[:], r1[:], rt[:])
                        else:
                            nc.vector.tensor_copy(r1[:], rt[:])
                            have_r1 = True
                    gd = posp.tile([128, 4, DF], BF16, tag="posg")
                    nc.vector.tensor_mul(gd[:], e1nat[:, bsl, :], e2nat[:, bsl, :])
                    r3 = posp.tile([128, 4], F32, tag="post")
                    nc.vector.tensor_reduce(
                        out=r3[:],
                        in_=gd[:],
                        axis=mybir.AxisListType.X,
                        op=mybir.AluOpType.add,
                    )
                    if have_r1:
                        nc.vector.tensor_add(pos_all[:, bsl], r1[:], r3[:])
                    else:
                        nc.vector.tensor_copy(pos_all[:, bsl], r3[:])
                nc.sync.dma_start(out=pos_d.ap(), in_=pos_all[:])

                for t in range(NT):
                    neg_tile_2048(t, 1)

            nc.vector.tensor_reduce(
                out=se_all[:],
                in_=acc_all[:].rearrange("p (t g) -> p t g", t=NT),
                axis=mybir.AxisListType.X,
                op=mybir.AluOpType.add,
            )
            nc.sync.dma_start(out=se_d.ap(), in_=se_all[:])

    nc.compile()
    return nc


_BUILD_CACHE: dict = {}


def _get_nc(r_self: int, r_tgt: int):
    key = (r_self, r_tgt)
    if key not in _BUILD_CACHE:
        _BUILD_CACHE[key] = _build(r_self, r_tgt)
    return _BUILD_CACHE[key]


def _cnt_inv(r: int) -> np.ndarray:
    """(128, NT) tile of 1/count(j) per local row (j = row mod L)."""
    j = np.arange(L)
    if r > 0:
        cnt = (np.minimum(j + r + 1, L) - np.maximum(j - r, 0)).astype(np.float64)
    else:
        cnt = np.ones(L)
    cinv = (1.0 / cnt).astype(np.float32)
    rows = (np.arange(NLOC) % L)
    return np.ascontiguousarray(cinv[rows].reshape(NT, 128).T)


def kernel(**inputs):
    loss, _ = _run(inputs, trace=False)
    return loss


def _run(inputs, trace=False, trace_kwargs=None):
    import ml_dtypes

    bf16 = ml_dtypes.bfloat16
    feature1 = inputs["feature1"]
    feature2 = inputs["feature2"]
    W1a, b1a, W2a, b2a = inputs["W1a"], inputs["b1a"], inputs["W2a"], inputs["b2a"]
    W1b, b1b, W2b, b2b = inputs["W1b"], inputs["b1b"], inputs["W2b"], inputs["b2b"]
    f1 = np.ascontiguousarray(np.asarray(feature1, dtype=np.float32))
    f2 = np.ascontiguousarray(np.asarray(feature2, dtype=np.float32))
    r_self = int(np.asarray(inputs["positive_range_self"]))
    r_tgt = int(np.asarray(inputs["positive_range_tgt"]))
    share_tgt = r_tgt == r_self

    nc = _get_nc(r_self, r_tgt)

    x2t_full = np.ascontiguousarray(f2.reshape(N, DIN2).T.astype(bf16))  # (192, 8192)
    bpk = np.zeros((128, 6), dtype=np.float32)
    bpk[:, 0:2] = np.asarray(b1a, np.float32).reshape(2, 128).T
    bpk[:, 2] = np.asarray(b2a, np.float32)
    bpk[:, 3:5] = np.asarray(b1b, np.float32).reshape(2, 128).T
    bpk[:, 5] = np.asarray(b2b, np.float32)
    common = {
        "w1a": np.ascontiguousarray(np.asarray(W1a, np.float32).astype(bf16)),
        "w2a": np.ascontiguousarray(np.asarray(W2a, np.float32).astype(bf16)),
        "w1b": np.ascontiguousarray(np.asarray(W1b, np.float32).astype(bf16)),
        "w2b": np.ascontiguousarray(np.asarray(W2b, np.float32).astype(bf16)),
        "bpk": bpk,
        "cis": _cnt_inv(r_self),
    }
    if r_tgt > 0 and not share_tgt:
        common["cit"] = _cnt_inv(r_tgt)

    in_maps = []
    for c in range(NCORES):
        x1t = np.ascontiguousarray(
            f1[c * NB : (c + 1) * NB].reshape(NLOC, DIN1).T.astype(bf16)
        )  # (256, 1024)
        # rotate feature2^T columns so this core's rows come first
        x2t = np.ascontiguousarray(
            np.concatenate(
                [x2t_full[:, c * NLOC :], x2t_full[:, : c * NLOC]], axis=1
            )
        )
        in_maps.append({**common, "x1t": x1t, "x2t": x2t})

    res = run_bass_kernel_spmd(
        nc,
        in_maps,
        core_ids=list(range(NCORES)),
        trace=trace,
        **(trace_kwargs or {}),
    )

    pos = np.empty(N, dtype=np.float64)
    se = np.empty(N, dtype=np.float64)
    for c in range(NCORES):
        # column t holds local rows [t*128, (t+1)*128) in partitions
        p = res.results[c]["pos_out"]  # (128, NT)
        s = res.results[c]["se_out"]
        pos[c * NLOC : (c + 1) * NLOC] = p.T.reshape(NLOC)
        se[c * NLOC : (c + 1) * NLOC] = s.T.reshape(NLOC)

    neg = np.log(se) - np.log(float(N))
    loss = np.mean(-pos + neg)
    return np.array(loss, dtype=np.float32), res
